# revision 17
# baseline (speedup 1.0000x reference)
"""Bert-BiLSTM-CRF Viterbi decode kernel for Trainium2 (8 NeuronCores, SPMD).

Problem: feats = einsum('bth,kh->btk', embeds, W_fc) + b_fc; Viterbi scan over
T=512 with argmax backtracking. B=64, T=512, H=768, K=11.

Sharding: data-parallel over batch B across 8 cores (8 batch elems/core).

Per-core algorithm (partitions p = c*8 + b, C=16 chunks x L=32 steps, reduced
tag space R=10 = {0..8, 10} since START=9 is never a predecessor for t>=2 and
row 9 is never on the path for t>=1; t=1 has the closed form psi_1 == 9,
delta_1 = trans[:, 9] + feat_1):
  FC     : PE transposes embeds tiles (fp32) + fp32r matmuls -> feats^T [11, b*T]
  M_all  : M_t[i,j] = trans[i,j] + feat_t[i]   (chunk matrices input)
  phase A: per-chunk max-plus matrix products A_c (parallel over (c,b))
  phase B: sequential boundary deltas through the 16 chunk matrices
  phase C: exact in-chunk recompute of deltas (ref fp32 op order)
  phase D: psi for all t in parallel (argmax via max + first-match-index)
  backtrk: in-chunk backpointer table composition + sequential chunk-boundary
           chain + parallel apply
Validated against the reference in fp32 numpy emulation: path exact, score
rel err ~1e-6.
"""
import sys
for _p in ("/opt/trn_rl_repo",):
    if _p not in sys.path:
        sys.path.append(_p)

import numpy as np
from contextlib import ExitStack

import concourse.bass as bass
import concourse.tile as tile
from concourse import bacc
from concourse import mybir

f32 = np.float32
B, T, H, K = 64, 512, 768, 11
NCORES = 8
BPC = B // NCORES          # batch per core = 8
C, L = 16, 32              # chunks x chunk length
R = 10                     # reduced tag count
KEEP = [0, 1, 2, 3, 4, 5, 6, 7, 8, 10]
START = 9
NEG = f32(-10000.0)
DT = mybir.dt.float32


def _consts():
    trans = None  # placeholder; constants that depend on transitions built in kernel()
    iota100 = np.tile(np.arange(R, dtype=f32), R)[None, :]            # (x,y)->y
    wdesc100 = np.repeat(np.arange(R, 0, -1, dtype=f32), 1)           # j -> R-j
    wdesc100 = np.tile((R - np.arange(R, dtype=f32)), R)[None, :]     # (i,j)->R-j
    wdesc10 = (R - np.arange(R, dtype=f32))[None, :]                  # x -> R-x
    iota10 = np.arange(R, dtype=f32)[None, :]
    idm = np.full((R, R), f32(-1e30), f32)
    np.fill_diagonal(idm, f32(0.0))
    return iota100, wdesc100, wdesc10, iota10, idm.reshape(1, R * R)


def build_kernel(nc):
    """Emit the full per-core program into `nc` (a bass.Bass)."""
    # ---- DRAM I/O ----
    emb_d = nc.dram_tensor("embeds_s", [BPC, T, H], DT, kind="ExternalInput").ap()
    w_d = nc.dram_tensor("w_t", [H, K], DT, kind="ExternalInput").ap()
    bfc_d = nc.dram_tensor("bias_red", [1, R], DT, kind="ExternalInput").ap()
    id_d = nc.dram_tensor("ident", [128, 128], DT, kind="ExternalInput").ap()
    trr_d = nc.dram_tensor("trans_red", [1, R * R], DT, kind="ExternalInput").ap()
    trsc_d = nc.dram_tensor("trans_sc", [1, R], DT, kind="ExternalInput").ap()
    idm_d = nc.dram_tensor("idm", [1, R * R], DT, kind="ExternalInput").ap()
    io100_d = nc.dram_tensor("iota100", [1, R * R], DT, kind="ExternalInput").ap()
    wd100_d = nc.dram_tensor("wdesc100", [1, R * R], DT, kind="ExternalInput").ap()
    wd10_d = nc.dram_tensor("wdesc10", [1, R], DT, kind="ExternalInput").ap()
    io10_d = nc.dram_tensor("iota10", [1, R], DT, kind="ExternalInput").ap()
    score_o = nc.dram_tensor("score_s", [BPC, 1], DT, kind="ExternalOutput").ap()
    path_o = nc.dram_tensor("path_s", [BPC, T], mybir.dt.int32, kind="ExternalOutput").ap()
    # scratch DRAM for partition-remap roundtrips
    feats_dram = nc.dram_tensor("feats_scr", [K, BPC * T], DT).ap()
    a_dram = nc.dram_tensor("a_scr", [128 * R * R], DT).ap()
    bnd_dram = nc.dram_tensor("bnd_scr", [BPC * C * R], DT).ap()
    t32_dram = nc.dram_tensor("t32_scr", [128 * R], DT).ap()
    bcol_dram = nc.dram_tensor("bcol_scr", [BPC * C], DT).ap()

    with tile.TileContext(nc) as tc, ExitStack() as ctx:
        _emit(ctx, tc, nc, dict(
            emb=emb_d, w=w_d, bfc=bfc_d, trr=trr_d, trsc=trsc_d, idm=idm_d,
            io100=io100_d, wd100=wd100_d, wd10=wd10_d, io10=io10_d, ident=id_d,
            score=score_o, path=path_o, feats_dram=feats_dram, a_dram=a_dram,
            bnd_dram=bnd_dram, t32_dram=t32_dram, bcol_dram=bcol_dram))
    if not nc.is_finalized():
        nc.finalize()
    return nc


import os
KPHASE = int(os.environ.get("KPHASE", "99"))
FC32 = os.environ.get("FC32", "1") == "1"   # plain fp32 matmul (exact-er, 4 c/r)
MMDT = mybir.dt.float32 if FC32 else mybir.dt.float32r


def _emit(ctx, tc, nc, d):
    AX = mybir.AxisListType.X
    OP = mybir.AluOpType

    def _cut(pool, n):
        if KPHASE > n:
            return False
        z = pool.tile([128, L], mybir.dt.int32, name=f"zcut{n}")
        nc.vector.memset(z[:], 0)
        zf = pool.tile([8, 1], DT, name=f"zfcut{n}")
        nc.vector.memset(zf[:], 0.0)
        nc.sync.dma_start(d["score"][:], zf[:])
        dstz = d["path"][:].rearrange("b (c x) -> c b x", c=C)
        for c in range(C):
            nc.sync.dma_start(dstz[c], z[c * 8:(c + 1) * 8, :])
        return True
    con = ctx.enter_context(tc.tile_pool(name="con", bufs=1))
    fcp = ctx.enter_context(tc.tile_pool(name="fcp", bufs=3))
    psum = ctx.enter_context(tc.tile_pool(name="psum", bufs=2, space="PSUM"))
    fps_pool = ctx.enter_context(tc.tile_pool(name="fps", bufs=2, space="PSUM"))
    big = ctx.enter_context(tc.tile_pool(name="big", bufs=1))
    scr = ctx.enter_context(tc.tile_pool(name="scr", bufs=2))

    # ---- constants ----
    ident = con.tile([128, 128], DT)
    nc.gpsimd.dma_start(ident[:], d["ident"][:])
    trans_rep = con.tile([128, R * R], DT)
    nc.gpsimd.dma_start(trans_rep[:], d["trr"][:].broadcast_to((128, R * R)))
    iota100 = con.tile([128, R * R], DT)
    nc.gpsimd.dma_start(iota100[:], d["io100"][:].broadcast_to((128, R * R)))
    wdesc100 = con.tile([128, R * R], DT)
    nc.gpsimd.dma_start(wdesc100[:], d["wd100"][:].broadcast_to((128, R * R)))
    iota10 = con.tile([128, R], DT)
    nc.gpsimd.dma_start(iota10[:], d["io10"][:].broadcast_to((128, R)))
    wdesc10 = con.tile([128, R], DT)
    nc.gpsimd.dma_start(wdesc10[:], d["wd10"][:].broadcast_to((128, R)))
    idm8 = con.tile([8, R * R], DT)
    nc.gpsimd.dma_start(idm8[:], d["idm"][:].broadcast_to((8, R * R)))
    trsc8 = con.tile([8, R], DT)
    nc.gpsimd.dma_start(trsc8[:], d["trsc"][:].broadcast_to((8, R)))
    bias_rep = con.tile([128, R], DT)
    nc.gpsimd.dma_start(bias_rep[:], d["bfc"][:].broadcast_to((128, R)))

    # ---- W^T chunks: [H, K] -> 6 x [128, K] (fp32r-rounded via ACT copy) ----
    HC = H // 128
    wt_raw = con.tile([128, HC * K], DT)
    wt = [con.tile([128, K], DT, name=f"wt{i}") for i in range(HC)]
    for hc in range(HC):
        nc.gpsimd.dma_start(wt_raw[:, hc * K:(hc + 1) * K],
                            d["w"][hc * 128:(hc + 1) * 128, :])
        nc.scalar.copy(wt[hc][:].bitcast(MMDT),
                       wt_raw[:, hc * K:(hc + 1) * K])
    # warm PE's view of ident so E-transposes carry a single DMA wait
    pswarm = psum.tile([1, 1], DT)
    nc.tensor.transpose(pswarm[:], ident[0:1, 0:1], ident[0:1, 0:1])

    if _cut(con, 1):
        return
    # ---- FC: feats^T [K, BPC*T] ----
    feats_sb = big.tile([K, BPC * T], DT)
    TS = T // 128  # 4 t-subtiles per batch elem
    for bb in range(BPC):
        en = [fcp.tile([128, H], DT, name=f"en{i}") for i in range(TS)]
        for tsub in range(TS):
            nc.gpsimd.dma_start(en[tsub][:], d["emb"][bb, tsub * 128:(tsub + 1) * 128, :])
        fps = fps_pool.tile([K, T], DT)
        for hc in range(HC):
            ps = psum.tile([128, T], DT)
            for tsub in range(TS):
                nc.tensor.transpose(ps[:, tsub * 128:(tsub + 1) * 128],
                                    en[tsub][:, hc * 128:(hc + 1) * 128], ident[:])
            et = fcp.tile([128, T], DT, name="et")
            nc.scalar.copy(et[:].bitcast(MMDT), ps[:])
            nc.tensor.matmul(fps[:], wt[hc][:].bitcast(MMDT),
                             et[:].bitcast(MMDT),
                             start=(hc == 0), stop=(hc == HC - 1))
        nc.scalar.copy(feats_sb[:, bb * T:(bb + 1) * T], fps[:])

    if _cut(con, 2):
        return
    # ---- feats relayout -> feats_bc [(c,b) 128, (r, s) R*L] via DRAM ----
    nc.sync.dma_start(d["feats_dram"][:], feats_sb[:])
    feats_bc = big.tile([128, R * L], DT)
    fd = d["feats_dram"][:].rearrange("k (b c s) -> k b c s", b=BPC, c=C)
    fbc_v = feats_bc[:].rearrange("p (r s) -> p r s", r=R)
    for c in range(C):
        # src (b, r, s) -> dst partitions c*8..c*8+8, free (r, s)
        nc.sync.dma_start(fbc_v[c * 8:(c + 1) * 8, 0:9, :],
                          fd[0:9, :, c].transpose([1, 0, 2]))
        nc.sync.dma_start(fbc_v[c * 8:(c + 1) * 8, 9:10, :],
                          fd[10:11, :, c].transpose([1, 0, 2]))

    # bias add: feats += b_fc (broadcast over s) - the single fp32 add of ref
    bias_v = bias_rep[:].unsqueeze(2).broadcast_to((128, R, L))
    nc.vector.tensor_add(feats_bc[:].rearrange("p (r s) -> p r s", r=R),
                         feats_bc[:].rearrange("p (r s) -> p r s", r=R), bias_v)

    if _cut(con, 3):
        return
    # ---- M_all [(c,b), (s, i, j)] = trans + feat ----
    m_all = big.tile([128, L * R * R], DT)
    m_v = m_all[:].rearrange("p (s i j) -> p s i j", s=L, i=R)
    tr_v = trans_rep[:].rearrange("p (i j) -> p i j", i=R).unsqueeze(1)\
        .broadcast_to((128, L, R, R))
    fe_v = feats_bc[:].rearrange("p (r s) -> p r s", r=R).transpose([0, 2, 1])\
        .unsqueeze(3).broadcast_to((128, L, R, R))
    nc.vector.tensor_add(m_v, tr_v, fe_v)
    # chunk 0, s=0 and s=1 -> max-plus identity
    nc.vector.tensor_copy(m_all[0:8, 0:R * R], idm8[:])
    nc.vector.tensor_copy(m_all[0:8, R * R:2 * R * R], idm8[:])

    if _cut(con, 4):
        return
    # ---- phase A: chunk matrix products ----
    a_st = big.tile([128, R * R], DT)       # A[i,k] at i*R+k
    nc.vector.tensor_copy(a_st[:], m_all[:, 0:R * R])
    for s in range(1, L):
        tmpA = scr.tile([128, R * R * R], DT, name="tmpA")
        m_s = m_all[:, s * R * R:(s + 1) * R * R].rearrange("p (i j) -> p i j", i=R)\
            .unsqueeze(2).broadcast_to((128, R, R, R))          # (i, k*, j)
        a_v = a_st[:].rearrange("p (j k) -> p j k", j=R).unsqueeze(1)\
            .broadcast_to((128, R, R, R)).transpose([0, 1, 3, 2])  # (i*, k, j)
        nc.vector.tensor_add(tmpA[:].rearrange("p (i k j) -> p i k j", i=R, k=R), m_s, a_v)
        nc.vector.reduce_max(a_st[:].rearrange("p (i k) -> p i k", i=R),
                             tmpA[:].rearrange("p (ik j) -> p ik j", j=R), axis=AX)

    if _cut(con, 5):
        return
    # ---- phase B: boundary deltas (on b-partitions, c in free) ----
    nc.sync.dma_start(d["a_dram"][:].rearrange("(p f) -> p f", p=128), a_st[:])
    a2 = big.tile([8, C * R * R], DT)
    a2_src = d["a_dram"][:].rearrange("(c b f) -> b c f", c=C, b=BPC)
    nc.sync.dma_start(a2[:].rearrange("p (c f) -> p c f", c=C), a2_src)
    bounds = big.tile([8, C * R], DT)
    # delta_1 = trans[:,9] + feat_1 (feats_bc partitions 0..8 are (c=0, b))
    f1 = feats_bc[0:8, :].rearrange("p (r s) -> p r s", r=R)[:, :, 1].squeeze(2) \
        if False else feats_bc[0:8, :].rearrange("p (r s) -> p r s", r=R)[:, :, 1:2]
    nc.vector.tensor_add(bounds[:, 0:R].unsqueeze(2), trsc8[:].unsqueeze(2), f1)
    for c in range(C - 1):
        tmpB = scr.tile([8, R * R], DT, name="tmpB")
        a_c = a2[:, c * R * R:(c + 1) * R * R].rearrange("p (i k) -> p i k", i=R)
        d_b = bounds[:, c * R:(c + 1) * R].unsqueeze(1).broadcast_to((8, R, R))
        nc.vector.tensor_add(tmpB[:].rearrange("p (i k) -> p i k", i=R), a_c, d_b)
        nc.vector.reduce_max(bounds[:, (c + 1) * R:(c + 2) * R],
                             tmpB[:].rearrange("p (i k) -> p i k", i=R), axis=AX)
    # roundtrip bounds -> hist slot 0
    nc.sync.dma_start(d["bnd_dram"][:].rearrange("(p f) -> p f", p=BPC), bounds[:])
    hist = big.tile([128, (L + 1) * R], DT)
    bnd_src = d["bnd_dram"][:].rearrange("(b c e) -> c b e", b=BPC, c=C)
    nc.sync.dma_start(hist[:, 0:R], bnd_src)

    if _cut(con, 6):
        return
    # ---- phase C: exact in-chunk deltas ----
    mhist = big.tile([128, L * R], DT)
    for s in range(L):
        u = scr.tile([128, R * R], DT, name="u")
        d_bc = hist[:, s * R:(s + 1) * R].unsqueeze(1).broadcast_to((128, R, R))
        nc.vector.tensor_add(u[:].rearrange("p (i j) -> p i j", i=R),
                             trans_rep[:].rearrange("p (i j) -> p i j", i=R), d_bc)
        nc.vector.reduce_max(mhist[:, s * R:(s + 1) * R],
                             u[:].rearrange("p (i j) -> p i j", i=R), axis=AX)
        fs = feats_bc[:].rearrange("p (r s) -> p r s", r=R)[:, :, s:s + 1]
        nc.vector.tensor_add(hist[:, (s + 1) * R:(s + 2) * R].unsqueeze(2),
                             mhist[:, s * R:(s + 1) * R].unsqueeze(2), fs)
        if s == 1:
            # chunk-0 slot 2 := delta_1 (steps t=0, t=1 are special-cased)
            nc.vector.tensor_copy(hist[0:8, 2 * R:3 * R], bounds[:, 0:R])

    if _cut(con, 7):
        return
    # ---- phase D: psi for all t ----
    U = big.tile([128, L * R * R], DT)
    d_v = hist[:, 0:L * R].rearrange("p (s j) -> p s j", s=L).unsqueeze(2)\
        .broadcast_to((128, L, R, R))
    nc.vector.tensor_add(U[:].rearrange("p (s i j) -> p s i j", s=L, i=R), tr_v, d_v)
    EQ = scr.tile([128, L * R * R], DT, name="EQ", bufs=1)
    m_bv = mhist[:].rearrange("p (s i) -> p s i", s=L).unsqueeze(3)\
        .broadcast_to((128, L, R, R))
    nc.vector.tensor_tensor(EQ[:].rearrange("p (s i j) -> p s i j", s=L, i=R),
                            U[:].rearrange("p (s i j) -> p s i j", s=L, i=R),
                            m_bv, op=OP.is_equal)
    WV = U  # reuse U's slot via separate tile would need more SBUF; write into new tile
    WVt = scr.tile([128, L * R * R], DT, name="WVt", bufs=1)
    wd_v = wdesc100[:].rearrange("p (i j) -> p i j", i=R).unsqueeze(1)\
        .broadcast_to((128, L, R, R))
    nc.vector.tensor_tensor(WVt[:].rearrange("p (s i j) -> p s i j", s=L, i=R),
                            EQ[:].rearrange("p (s i j) -> p s i j", s=L, i=R),
                            wd_v, op=OP.mult)
    psi = big.tile([128, L * R], DT)
    nc.vector.reduce_max(psi[:].rearrange("p (s i) -> p s i", s=L),
                         WVt[:].rearrange("p (si j) -> p si j", j=R), axis=AX)
    nc.vector.tensor_scalar(psi[:], psi[:], -1.0, float(R),
                            op0=OP.mult, op1=OP.add)

    if _cut(con, 8):
        return
    # ---- backtrack composition: T_r tables ----
    tbuf = big.tile([128, L * R], DT)   # slice rho=r-1 holds T_r
    nc.vector.tensor_copy(tbuf[:, 0:R], psi[:, (L - 1) * R:L * R])
    for r in range(2, L + 1):
        s = L - r
        eqm = scr.tile([128, R * R], DT, name="eqm")
        t_prev = tbuf[:, (r - 2) * R:(r - 1) * R].unsqueeze(2).broadcast_to((128, R, R))
        nc.vector.tensor_tensor(eqm[:].rearrange("p (x y) -> p x y", x=R),
                                t_prev, iota100[:].rearrange("p (x y) -> p x y", x=R),
                                op=OP.is_equal)
        pr = scr.tile([128, R * R], DT, name="pr")
        psi_s = psi[:, s * R:(s + 1) * R].unsqueeze(1).broadcast_to((128, R, R))
        nc.vector.tensor_tensor(pr[:].rearrange("p (x y) -> p x y", x=R),
                                eqm[:].rearrange("p (x y) -> p x y", x=R), psi_s,
                                op=OP.mult)
        nc.vector.reduce_sum(tbuf[:, (r - 1) * R:r * R],
                             pr[:].rearrange("p (x y) -> p x y", x=R), axis=AX)

    if _cut(con, 9):
        return
    # ---- last tag + score (chunk 15 partitions 120..128 -> offset 0) ----
    fin8 = con.tile([8, R], DT)
    nc.sync.dma_start(fin8[:], hist[120:128, L * R:(L + 1) * R])
    score8 = con.tile([8, 1], DT)
    nc.vector.reduce_max(score8[:], fin8[:], axis=AX)
    nc.sync.dma_start(d["score"][:], score8[:])
    eql = con.tile([8, R], DT)
    nc.vector.tensor_scalar(eql[:], fin8[:], score8[:], None, op0=OP.is_equal)
    wvl = con.tile([8, R], DT)
    nc.vector.tensor_tensor(wvl[:], eql[:], wdesc10[0:8, :], op=OP.mult)
    lastr = con.tile([8, 1], DT)
    nc.vector.reduce_max(lastr[:], wvl[:], axis=AX)
    nc.vector.tensor_scalar(lastr[:], lastr[:], -1.0, float(R),
                            op0=OP.mult, op1=OP.add)

    if _cut(con, 10):
        return
    # ---- beta chain over chunks (b-partitions) ----
    nc.sync.dma_start(d["t32_dram"][:].rearrange("(p f) -> p f", p=128), tbuf[:, (L - 1) * R:L * R])
    t32 = con.tile([8, C * R], DT)
    t32_src = d["t32_dram"][:].rearrange("(c b e) -> b c e", c=C, b=BPC)
    nc.sync.dma_start(t32[:].rearrange("p (c e) -> p c e", c=C), t32_src)
    bcol = con.tile([8, C], DT)
    nc.vector.tensor_copy(bcol[:, C - 1:C], lastr[:])
    for c in range(C - 1, 0, -1):
        eqb = scr.tile([8, R], DT, name="eqb")
        nc.vector.tensor_scalar(eqb[:], iota10[0:8, :], bcol[:, c:c + 1], None,
                                op0=OP.is_equal)
        prb = scr.tile([8, R], DT, name="prb")
        nc.vector.tensor_tensor(prb[:], t32[:, c * R:(c + 1) * R], eqb[:], op=OP.mult)
        nc.vector.reduce_sum(bcol[:, c - 1:c], prb[:], axis=AX)
    nc.sync.dma_start(d["bcol_dram"][:].rearrange("(p f) -> p f", p=BPC), bcol[:])
    beta_bc = con.tile([128, 1], DT)
    for c in range(C):
        nc.sync.dma_start(beta_bc[c * 8:(c + 1) * 8, :],
                          d["bcol_dram"][:].rearrange("(b c) -> b c", b=BPC)[:, c:c + 1])

    if _cut(con, 11):
        return
    # ---- apply: path values ----
    eqa = con.tile([128, R], DT)
    nc.vector.tensor_scalar(eqa[:], iota10[:], beta_bc[:], None, op0=OP.is_equal)
    pra = scr.tile([128, L * R], DT, name="pra", bufs=1)
    eqa_v = eqa[:].unsqueeze(1).broadcast_to((128, L, R))
    nc.vector.tensor_tensor(pra[:].rearrange("p (r x) -> p r x", r=L),
                            tbuf[:].rearrange("p (r x) -> p r x", r=L), eqa_v,
                            op=OP.mult)
    pv = big.tile([128, L], DT)   # slot 0 = beta (t=32c+31), slots 1..31 from T_r
    nc.vector.tensor_copy(pv[:, 0:1], beta_bc[:])
    nc.vector.reduce_sum(pv[:, 1:L].unsqueeze(2),
                         pra[:, 0:(L - 1) * R].rearrange("p (r x) -> p r x", x=R),
                         axis=AX)
    # map reduced -> real tags: real = x + (x == 9)
    eq9 = scr.tile([128, L], DT, name="eq9")
    nc.vector.tensor_scalar(eq9[:], pv[:], 9.0, None, op0=OP.is_equal)
    nc.vector.tensor_add(pv[:], pv[:], eq9[:])
    # path[0] = START
    nc.vector.memset(pv[0:8, L - 1:L], 9.0)
    pvi = big.tile([128, L], mybir.dt.int32)
    # reverse along free dim during the cast so DMA strides stay positive:
    # pvi[:, x] = pv[:, L-1-x]  -> pvi slot x corresponds to t = 32c + x
    nc.vector.tensor_copy(pvi[:], pv[:][:, ::-1])
    dst = d["path"][:].rearrange("b (c x) -> c b x", c=C)
    for c in range(C):
        nc.sync.dma_start(dst[c], pvi[c * 8:(c + 1) * 8, :])


def kernel(embeds, W_fc, b_fc, transitions):
    """Full-input entry point: shard over 8 cores, run SPMD, gather."""
    from concourse.bass_utils import run_bass_kernel_spmd
    keep = np.array(KEEP)

    embeds = np.ascontiguousarray(embeds, dtype=f32)
    W_fc = np.ascontiguousarray(W_fc, dtype=f32)
    b_fc = np.ascontiguousarray(b_fc, dtype=f32)
    transitions = np.ascontiguousarray(transitions, dtype=f32)

    trans_red = transitions[np.ix_(keep, keep)].astype(f32).reshape(1, R * R)
    trans_sc = transitions[keep, START].astype(f32).reshape(1, R)
    iota100, wdesc100, wdesc10, iota10, idm = _consts()

    nc = bacc.Bacc()
    build_kernel(nc)

    base = {
        "w_t": np.ascontiguousarray(W_fc.T), "bias_red": b_fc[keep].reshape(1, R),
        "ident": np.eye(128, dtype=f32), "trans_red": trans_red,
        "trans_sc": trans_sc, "idm": idm, "iota100": iota100,
        "wdesc100": wdesc100, "wdesc10": wdesc10, "iota10": iota10,
    }
    in_maps = [dict(base, embeds_s=embeds[i * BPC:(i + 1) * BPC]) for i in range(NCORES)]
    res = run_bass_kernel_spmd(nc, in_maps, core_ids=list(range(NCORES)))
    score = np.concatenate([r["score_s"].reshape(BPC) for r in res.results])
    path = np.concatenate([r["path_s"].reshape(BPC, T) for r in res.results])
    return score.astype(f32), path.astype(np.int32)


if __name__ == "__main__":
    ins = {k: np.load(f"/tmp/{k}.npy") for k in ["embeds", "W_fc", "b_fc", "transitions"]}
    s, p = kernel(**ins)
    print("score[:4]:", s[:4])
    print("path[0, :8]:", p[0, :8])
